# revision 1
# baseline (speedup 1.0000x reference)
"""Trainium2 Bass kernel for nn_Conv_39273180955618.

The reference op reduces to a depthwise correlation: every image (batch x
channel plane) of X is correlated with the same 3x3 kernel
Keff = K.sum((0,1)), plus a scalar bias b * prod(K.shape).

Strategy (8 NeuronCores, data-parallel over batch):
  - core k gets batches [2k, 2k+2) = 128 images of 224x224.
  - Per core, images are processed in blocks of IB images x 112-row chunks.
    Rows live on SBUF partitions, W stays contiguous on the free axis.
  - The H-convolution is a TensorE matmul contraction over rows with small
    banded matrices B[chunk, dw] (shape [113, 112]): for each of the 3 W
    shifts dw, Z[:, wout] += B^T @ X[rows, win], accumulated in PSUM.
    H zero-padding is folded into the band matrices, W zero-padding into
    the matmul column ranges.
  - fp32 data is fed to the PE as float32r (full-rate fp32 matmul mode).
  - PSUM -> SBUF eviction (+ bias) alternates between ScalarE and VectorE,
    and DMA in/out transfers are ~1.6 MB each for near-peak HBM bandwidth.
"""

import numpy as np

import bass_rust
import concourse.bass as bass
import concourse.mybir as mybir
import concourse.tile as tile
from concourse.bass_utils import run_bass_kernel_spmd

F32 = mybir.dt.float32
F32R = mybir.dt.float32r

N_CORES = 8
H = W = 224
M = 112        # output rows per chunk
KR = 113       # input rows per chunk (M + 1 halo row at the image edge)
IMGS = 128     # images per core (2 batches x 64 channels)
IB = 32        # images per block (DMA granularity)
NBLK = IMGS // IB
WP = W + 2     # padded image-row width in SBUF (zero column at each edge)
NWIN = 2 * WP - 2  # flat matmul window: 2 images per PSUM group, minus 2
# (r0, i0) per chunk: output-row base and input-row base.
CHUNKS = ((0, 0), (112, 111))

_MAX_WAITS = 1


def _split_multi_waits(nc):
    """Split instructions carrying >1 sync-wait into single-wait NOP
    preludes (the walrus build here rejects multi-wait instructions)."""
    counter = 0
    for fn in nc.m.functions:
        for bb in fn.blocks:
            insts = bb.instructions
            i = 0
            while i < len(insts):
                inst = insts[i]
                si = inst.sync_info
                if si is not None and si.on_wait and len(si.on_wait) > _MAX_WAITS:
                    waits = list(si.on_wait)
                    keep = waits[-_MAX_WAITS:]
                    spill = waits[:-_MAX_WAITS]
                    nops = []
                    for w in spill:
                        nop = mybir.InstNoOp(
                            name=f"waitsplit_{counter}", ins=[], outs=[]
                        )
                        counter += 1
                        nop.engine = inst.engine
                        nop.sync_info = bass_rust.SyncInfo(on_wait=[w], on_update=[])
                        nops.append(nop)
                    inst.sync_info = bass_rust.SyncInfo(
                        on_wait=keep,
                        on_update=list(si.on_update) if si.on_update else [],
                    )
                    insts[i:i] = nops
                    i += len(nops)
                i += 1
    return counter


def build_nc(bias_total: float):
    nc = bass.Bass("TRN2", target_bir_lowering=False, debug=False)
    x_d = nc.dram_tensor("X", [IMGS, H, WP], F32R, kind="ExternalInput").ap()
    bands_d = nc.dram_tensor("BANDS", [2, 3, KR, M], F32R, kind="ExternalInput").ap()
    y_d = nc.dram_tensor("Y", [IMGS, H, W], F32, kind="ExternalOutput").ap()

    with tile.TileContext(nc) as tc:
        with (
            tc.tile_pool(name="const", bufs=1) as cpool,
            tc.tile_pool(name="io", bufs=3) as io_pool,
            tc.tile_pool(name="acc", bufs=8, space="PSUM") as psum_pool,
        ):
            bands = cpool.tile([KR, 2, 3, M], F32R)
            nc.sync.dma_start(bands, bands_d.rearrange("c s k m -> k c s m"))
            ev = 0
            for blk in range(NBLK):
                for c, (r0, i0) in enumerate(CHUNKS):
                    xt = io_pool.tile([KR, IB, WP], F32R, tag="xt")
                    # X arrives host-padded to 226 columns (zero at each
                    # edge), so the DMA delivers the W padding directly.
                    nc.sync.dma_start(
                        xt,
                        x_d[blk * IB:(blk + 1) * IB, i0:i0 + KR, :].rearrange(
                            "i r w -> r i w"
                        ),
                    )
                    xtf = xt.rearrange("k i w -> k (i w)")
                    ot = io_pool.tile([M, IB, W], F32, tag="ot")
                    for p in range(IB // 2):
                        base = 2 * p * WP
                        # One flat 450-wide window per W-shift: fp32r matmuls
                        # need a single even-count free dim and an 8B-aligned
                        # PSUM dst at offset 0, so the dst is always [:, 0:450]
                        # and the W-shift slides the source window. PSUM
                        # columns 224/225 catch the inter-image junk and are
                        # not evicted.
                        ps = psum_pool.tile([M, 2 * WP], F32)
                        for k, dw in enumerate((0, 1, 2)):
                            nc.tensor.matmul(
                                ps[:, 0:NWIN],
                                bands[:, c, dw, :],
                                xtf[:, base + dw:base + dw + NWIN],
                                start=(k == 0),
                                stop=(k == 2),
                            )
                        psv = ps.rearrange("m (i w) -> m i w", w=WP)[:, :, 0:W]
                        dst = ot[:, 2 * p:2 * p + 2, :]
                        if ev % 2 == 0:
                            if bias_total != 0.0:
                                nc.scalar.activation(
                                    dst,
                                    psv,
                                    mybir.ActivationFunctionType.Copy,
                                    bias=float(bias_total),
                                )
                            else:
                                nc.scalar.copy(dst, psv)
                        else:
                            if bias_total != 0.0:
                                nc.vector.tensor_scalar_add(
                                    dst, psv, float(bias_total)
                                )
                            else:
                                nc.vector.tensor_copy(dst, psv)
                        ev += 1
                        # Stores go on the ACT HWDGE ring so the next
                        # block's load (SP ring) never queues behind this
                        # store's eviction wait; two half-stores per block
                        # let the store pipeline start after 4 evictions.
                        if p % 4 == 3:
                            h0 = (p - 3) * 2
                            nc.scalar.dma_start(
                                y_d[
                                    blk * IB + h0:blk * IB + h0 + 8,
                                    r0:r0 + M,
                                    :,
                                ].rearrange("i r w -> r i w"),
                                ot[:, h0:h0 + 8, :],
                            )
    _split_multi_waits(nc)
    return nc


def build_bands(Keff: np.ndarray) -> np.ndarray:
    """Banded H-contraction matrices, [chunk, dw, KR, M] fp32.

    B[c, dw, i, m] = Keff[dh, dw] where input-row index i corresponds to
    absolute row i0 + i and output row r0 + m needs absolute row
    r0 + m + dh - 1; rows outside [0, H) are dropped (zero padding).
    """
    bands = np.zeros((2, 3, KR, M), dtype=np.float32)
    for c, (r0, i0) in enumerate(CHUNKS):
        for dw in range(3):
            for m in range(M):
                for dh in range(3):
                    arow = r0 + m + dh - 1
                    if 0 <= arow < H:
                        bands[c, dw, arow - i0, m] = Keff[dh, dw]
    return bands


_cache = {}


def kernel(X, K, b, padding, stride) -> np.ndarray:
    X = np.ascontiguousarray(np.asarray(X, dtype=np.float32))
    K = np.asarray(K, dtype=np.float32)
    b = np.asarray(b, dtype=np.float32)
    assert int(padding) == 1 and int(stride) == 1, (padding, stride)
    bx, cx, hx, wx = X.shape
    assert (bx, cx, hx, wx) == (16, 64, H, W), X.shape

    bk, ck, hk, wk = K.shape
    Keff = K.sum(axis=(0, 1), dtype=np.float32)
    bias_total = float(b.reshape(())) * (bk * ck * hk * wk)

    key = (round(bias_total, 12) != 0.0)
    if key not in _cache:
        _cache[key] = build_nc(bias_total)
    nc = _cache[key]

    bands = build_bands(Keff)
    Xf = X.reshape(bx * cx, hx, wx)
    Xp = np.zeros((bx * cx, hx, WP), dtype=np.float32)
    Xp[:, :, 1:1 + W] = Xf
    in_maps = [
        {
            "X": Xp[k * IMGS:(k + 1) * IMGS],
            "BANDS": bands,
        }
        for k in range(N_CORES)
    ]
    res = run_bass_kernel_spmd(nc, in_maps, core_ids=list(range(N_CORES)))
    out = np.concatenate([r["Y"] for r in res.results], axis=0)
    return out.reshape(bx, cx, hx, wx)



# revision 2
# speedup vs baseline: 1.8410x; 1.8410x over previous
"""Trainium2 Bass kernel for nn_Conv_39273180955618 — fp16 I/O version.

The reference op reduces to a depthwise correlation: every image (batch x
channel plane) of X is correlated with the same 3x3 kernel
Keff = K.sum((0,1)), plus a scalar bias b * prod(K.shape).

The fp32 baseline was DMA-bound (51.4 MB/core = 144 us at the 360 GB/s
DMA roofline, 97% busy).  This version moves all HBM traffic to fp16
(25.9 MB/core, ~72.4 us) which lands DMA and PE (~71.7 us of fp16
matmuls at 2.4 GHz) on a balanced roofline:

  - Host prepares X as [4 img-blocks, 224 rows, 226 cols, 32 imgs] fp16
    per core (W zero-pad baked in).  Keeping a 32-image block innermost
    makes every DMA descriptor a contiguous multi-KB run (far above the
    512 B threshold below which DMA pays a 2x latency penalty) AND makes
    every matmul window shift a multiple of 32 elements = 64 B.  fp16
    matmul operands must be 4-byte aligned on real silicon — a plain
    (img, w) layout with odd dw-shift offsets (2 B) crashes the device.
  - Input rows live on SBUF partitions; the H-convolution is a banded
    matmul contraction (zero rows at the image top/bottom edges are
    dropped inside the band).  Each PSUM tile is [112 out-rows,
    16 w-positions x 32 imgs = 512] fp32, accumulated by 3 matmuls (one
    per W shift, rhs window offset (16t+dw)*32).  fp16 matmuls cost
    1 cycle/row in the PE cost model, same as fp32r; PSUM stays fp32.
  - PSUM -> SBUF eviction (+ bias) converts to fp16, alternating
    ScalarE / VectorE; stores go out on the otherwise-idle GPSIMD
    (SWDGE) ring so store waits never head-of-line-block the eviction
    engines or the load ring.
  - Host transposes Y back to [imgs, rows, cols] and upcasts to fp32.
  - The PE p-state ramp (0.65 -> 1.2 -> 2.4 GHz over ~3 us of busy
    time, reset by multi-us idles) is absorbed before real work by a
    chain of 2-column warm-up matmuls paced ~0.7 us apart by VectorE
    copies of a memset tile — no DMA dependency, so the ramp runs out
    while the bands + first X sub-tile are still loading.
  - The first chunk is loaded in 4 column-range sub-tiles so the first
    real matmul starts ~4.4 us in; the last chunk's tail stores are
    16-column slices on alternating rings so the final store chases the
    final eviction by ~0.3 us.

fp16 keeps 10 mantissa bits: worst-case elementwise error ~0.1-0.2% of
|Z|, i.e. ~1.5 absolute vs the 22 allowed by the 2e-2 gate.
"""

import numpy as np

import bass_rust
import concourse.bass as bass
import concourse.mybir as mybir
import concourse.tile as tile
from concourse.bass_utils import run_bass_kernel_spmd

F32 = mybir.dt.float32
F16 = mybir.dt.float16

N_CORES = 8
H = W = 224
M = 112        # output rows per chunk
KR = 113       # input rows per chunk (M + 1 halo row at the image edge)
IMGS = 128     # images per core (2 batches x 64 channels)
IBLK = 32      # images per DRAM block (innermost layout dim)
NBLK = IMGS // IBLK
WP = W + 2     # padded image-row width (zero column at each edge)
WT = 16        # output w-positions per PSUM tile
NT = W // WT   # PSUM tiles per block-chunk
PSF = WT * IBLK  # PSUM free size (512 fp32 = one full bank)
# (r0, i0) per chunk: output-row base and input-row base.
CHUNKS = ((0, 0), (112, 111))
NWARM = 6      # paced warm-up matmuls covering t ~ [1 us, 5.5 us]
PACE_F = 600   # VectorE pacing-copy length: ~625 ns per link
# First-chunk column-range sub-loads: tile t needs cols <= 16t+17.
SUBS0 = ((0, 34), (34, 66), (66, 130), (130, WP))

_MAX_WAITS = 1


def _split_multi_waits(nc):
    """Split instructions carrying >1 sync-wait into single-wait NOP
    preludes (the walrus build here rejects multi-wait instructions)."""
    counter = 0
    for fn in nc.m.functions:
        for bb in fn.blocks:
            insts = bb.instructions
            i = 0
            while i < len(insts):
                inst = insts[i]
                si = inst.sync_info
                if si is not None and si.on_wait and len(si.on_wait) > _MAX_WAITS:
                    waits = list(si.on_wait)
                    keep = waits[-_MAX_WAITS:]
                    spill = waits[:-_MAX_WAITS]
                    nops = []
                    for w in spill:
                        nop = mybir.InstNoOp(
                            name=f"waitsplit_{counter}", ins=[], outs=[]
                        )
                        counter += 1
                        nop.engine = inst.engine
                        nop.sync_info = bass_rust.SyncInfo(on_wait=[w], on_update=[])
                        nops.append(nop)
                    inst.sync_info = bass_rust.SyncInfo(
                        on_wait=keep,
                        on_update=list(si.on_update) if si.on_update else [],
                    )
                    insts[i:i] = nops
                    i += len(nops)
                i += 1
    return counter


def build_nc(bias_total: float):
    nc = bass.Bass("TRN2", target_bir_lowering=False, debug=False)
    x_d = nc.dram_tensor("X", [NBLK, H, WP, IBLK], F16, kind="ExternalInput").ap()
    # Bands pre-transposed on host to [KR, chunk, dw, M] so the load is a
    # straight descriptor-per-partition copy.
    bands_d = nc.dram_tensor("BANDS", [KR, 2, 3, M], F16, kind="ExternalInput").ap()
    y_d = nc.dram_tensor("Y", [NBLK, H, W, IBLK], F16, kind="ExternalOutput").ap()

    with tile.TileContext(nc) as tc:
        with (
            tc.tile_pool(name="const", bufs=1) as cpool,
            tc.tile_pool(name="xin", bufs=5) as xin_pool,
            tc.tile_pool(name="out", bufs=4) as out_pool,
            tc.tile_pool(name="acc", bufs=8, space="PSUM") as psum_pool,
        ):
            # --- PE p-state warm-up, data-independent (memset-fed) -----
            # A tiny first memset gets warm-up 0 onto the PE ~0.5 us in
            # (pinning pe_busy_start early); the remaining warm-ups are
            # paced ~0.7 us apart by VectorE copies so the busy-run never
            # looks idle long enough to reset the ramp before real work.
            pace0 = cpool.tile([KR, M + 2], F16)
            nc.vector.memset(pace0, 0.0)
            wps = psum_pool.tile([M, PSF], F32, tag="ps")
            nc.tensor.matmul(
                wps[:, 0:2], pace0[:, 0:M], pace0[:, 0:2],
                start=True, stop=True,
            )
            pace = cpool.tile([KR, NWARM, PACE_F], F16)
            nc.vector.memset(pace[:, 0, :], 0.0)
            for j in range(NWARM):
                if j > 0:
                    nc.vector.tensor_copy(pace[:, j, :], pace[:, j - 1, :])
                wps = psum_pool.tile([M, PSF], F32, tag="ps")
                nc.tensor.matmul(
                    wps[:, 0:2], pace[:, j, 0:M], pace[:, j, 0:2],
                    start=True, stop=True,
                )

            bands = cpool.tile([KR, 2, 3, M], F16)

            ev = 0
            for bi in range(NBLK):
                for c, (r0, i0) in enumerate(CHUNKS):
                    first = bi == 0 and c == 0
                    is_last = bi == NBLK - 1 and c == len(CHUNKS) - 1
                    xt = xin_pool.tile([KR, WP, IBLK], F16, tag="xt")
                    for wlo, whi in SUBS0 if first else ((0, WP),):
                        nc.sync.dma_start(
                            xt[:, wlo:whi, :],
                            x_d[bi, i0:i0 + KR, wlo:whi, :],
                        )
                        if first and wlo == 0:
                            # Bands on the ACT ring right after the first
                            # X sub-load: their DGE setups overlap.
                            nc.scalar.dma_start(bands, bands_d)
                    xtf = xt.rearrange("k w i -> k (w i)")
                    ot = out_pool.tile([M, W, IBLK], F16, tag="ot")
                    otf = ot.rearrange("m w i -> m (w i)")
                    for t in range(NT):
                        ps = psum_pool.tile([M, PSF], F32, tag="ps")
                        for k, dw in enumerate((0, 1, 2)):
                            nc.tensor.matmul(
                                ps,
                                bands[:, c, dw, :],
                                xtf[:, (WT * t + dw) * IBLK:
                                    (WT * t + dw) * IBLK + PSF],
                                start=(k == 0),
                                stop=(k == 2),
                            )
                        dst = otf[:, t * PSF:(t + 1) * PSF]
                        if ev % 2 == 0:
                            if bias_total != 0.0:
                                nc.scalar.activation(
                                    dst,
                                    ps,
                                    mybir.ActivationFunctionType.Copy,
                                    bias=float(bias_total),
                                )
                            else:
                                nc.scalar.copy(dst, ps)
                        else:
                            if bias_total != 0.0:
                                nc.vector.tensor_scalar_add(
                                    dst, ps, float(bias_total)
                                )
                            else:
                                nc.vector.tensor_copy(dst, ps)
                        ev += 1
                        # Stores ride the otherwise-idle GPSIMD (SWDGE)
                        # ring so their sem waits never block the load
                        # ring or the eviction engines.  The last chunk
                        # drains in 16-column slices on alternating rings
                        # so the final store chases the final eviction.
                        if is_last and t >= 8:
                            wlo = WT * t
                            ring = (
                                nc.sync if t == NT - 1
                                else nc.scalar if t == NT - 2
                                else nc.gpsimd
                            )
                            ring.dma_start(
                                y_d[bi, r0:r0 + M, wlo:wlo + WT, :],
                                ot[:, wlo:wlo + WT, :],
                            )
                        elif t in (3, 7, 10, 13) and not (is_last and t > 7):
                            wlo = {3: 0, 7: 64, 10: 128, 13: 176}[t]
                            whi = {3: 64, 7: 128, 10: 176, 13: 224}[t]
                            nc.gpsimd.dma_start(
                                y_d[bi, r0:r0 + M, wlo:whi, :],
                                ot[:, wlo:whi, :],
                            )
    _split_multi_waits(nc)
    return nc


def build_bands(Keff: np.ndarray) -> np.ndarray:
    """Banded H-contraction matrices, [KR, chunk, dw, M] fp16.

    B[i, c, dw, m] = Keff[dh, dw] where input-row index i corresponds to
    absolute row i0 + i and output row r0 + m needs absolute row
    r0 + m + dh - 1; rows outside [0, H) are dropped (zero padding).
    """
    bands = np.zeros((2, 3, KR, M), dtype=np.float32)
    for c, (r0, i0) in enumerate(CHUNKS):
        for dw in range(3):
            for m in range(M):
                for dh in range(3):
                    arow = r0 + m + dh - 1
                    if 0 <= arow < H:
                        bands[c, dw, arow - i0, m] = Keff[dh, dw]
    return np.ascontiguousarray(bands.transpose(2, 0, 1, 3)).astype(np.float16)


_cache = {}


def kernel(X, K, b, padding, stride) -> np.ndarray:
    X = np.asarray(X, dtype=np.float32)
    K = np.asarray(K, dtype=np.float32)
    b = np.asarray(b, dtype=np.float32)
    assert int(padding) == 1 and int(stride) == 1, (padding, stride)
    bx, cx, hx, wx = X.shape
    assert (bx, cx, hx, wx) == (16, 64, H, W), X.shape

    bk, ck, hk, wk = K.shape
    Keff = K.sum(axis=(0, 1), dtype=np.float32)
    bias_total = float(b.reshape(())) * (bk * ck * hk * wk)

    key = (round(bias_total, 12) != 0.0)
    if key not in _cache:
        _cache[key] = build_nc(bias_total)
    nc = _cache[key]

    bands = build_bands(Keff)
    # Host-side prep: fp16, zero-pad W to 226, lay out per core as
    # [img-block, row, col, img-in-block] so DMA descriptors are
    # contiguous multi-KB runs and matmul window shifts are 64 B-aligned.
    Xf = X.reshape(bx * cx, hx, wx)
    Xp = np.zeros((bx * cx, hx, WP), dtype=np.float16)
    Xp[:, :, 1:1 + W] = Xf
    Xp = Xp.reshape(N_CORES, NBLK, IBLK, hx, WP)
    in_maps = [
        {
            "X": np.ascontiguousarray(Xp[k].transpose(0, 2, 3, 1)),
            "BANDS": bands,
        }
        for k in range(N_CORES)
    ]
    res = run_bass_kernel_spmd(nc, in_maps, core_ids=list(range(N_CORES)))
    # Y comes back [blk, row, col, img] per core -> [imgs, row, col].
    out = np.concatenate(
        [
            np.asarray(r["Y"]).transpose(0, 3, 1, 2).reshape(IMGS, hx, wx)
            for r in res.results
        ],
        axis=0,
    )
    return np.ascontiguousarray(out).astype(np.float32).reshape(bx, cx, hx, wx)


# revision 14
# speedup vs baseline: 1.8498x; 1.0048x over previous
"""Trainium2 Bass kernel for nn_Conv_39273180955618 — fp16 I/O version.

The reference op reduces to a depthwise correlation: every image (batch x
channel plane) of X is correlated with the same 3x3 kernel
Keff = K.sum((0,1)), plus a scalar bias b * prod(K.shape).

The fp32 baseline was DMA-bound (51.4 MB/core = 144 us at the 360 GB/s
DMA roofline, 97% busy).  This version moves all HBM traffic to fp16
(25.9 MB/core, ~72.4 us) which lands DMA and PE (~71.7 us of fp16
matmuls at 2.4 GHz) on a balanced roofline:

  - Host prepares X as [4 img-blocks, 224 rows, 226 cols, 32 imgs] fp16
    per core (W zero-pad baked in).  Keeping a 32-image block innermost
    makes every DMA descriptor a contiguous multi-KB run (far above the
    512 B threshold below which DMA pays a 2x latency penalty) AND makes
    every matmul window shift a multiple of 32 elements = 64 B.  fp16
    matmul operands must be 4-byte aligned on real silicon — a plain
    (img, w) layout with odd dw-shift offsets (2 B) crashes the device.
  - Input rows live on SBUF partitions; the H-convolution is a banded
    matmul contraction (zero rows at the image top/bottom edges are
    dropped inside the band).  Each PSUM tile is [112 out-rows,
    16 w-positions x 32 imgs = 512] fp32, accumulated by 3 matmuls (one
    per W shift, rhs window offset (16t+dw)*32).  fp16 matmuls cost
    1 cycle/row in the PE cost model, same as fp32r; PSUM stays fp32.
  - PSUM -> SBUF eviction (+ bias) converts to fp16, alternating
    ScalarE / VectorE; stores go out on the otherwise-idle GPSIMD
    (SWDGE) ring so store waits never head-of-line-block the eviction
    engines or the load ring.
  - Host transposes Y back to [imgs, rows, cols] and upcasts to fp32.
  - The PE p-state ramp (0.65 -> 1.2 -> 2.4 GHz over ~3 us of busy
    time, reset by multi-us idles) is absorbed before real work by a
    chain of 2-column warm-up matmuls paced ~0.7 us apart by VectorE
    copies of a memset tile — no DMA dependency, so the ramp runs out
    while the bands + first X sub-tile are still loading.
  - The first chunk is loaded in 4 column-range sub-tiles so the first
    real matmul starts ~4.4 us in; the last chunk's tail stores are
    16-column slices on alternating rings so the final store chases the
    final eviction by ~0.3 us.

fp16 keeps 10 mantissa bits: worst-case elementwise error ~0.1-0.2% of
|Z|, i.e. ~1.5 absolute vs the 22 allowed by the 2e-2 gate.
"""

import numpy as np

import bass_rust
import concourse.bass as bass
import concourse.mybir as mybir
import concourse.tile as tile
from concourse.bass_utils import run_bass_kernel_spmd

F32 = mybir.dt.float32
F16 = mybir.dt.float16

N_CORES = 8
H = W = 224
M = 112        # output rows per chunk
KR = 113       # input rows per chunk (M + 1 halo row at the image edge)
IMGS = 128     # images per core (2 batches x 64 channels)
IBLK = 32      # images per DRAM block (innermost layout dim)
NBLK = IMGS // IBLK
WP = W + 2     # padded image-row width (zero column at each edge)
WT = 16        # output w-positions per PSUM tile
NT = W // WT   # PSUM tiles per block-chunk
PSF = WT * IBLK  # PSUM free size (512 fp32 = one full bank)
# (r0, i0) per chunk: output-row base and input-row base.
CHUNKS = ((0, 0), (112, 111))
NWARM = 6      # paced warm-up matmuls covering t ~ [1 us, 5.5 us]
PACE_F = 600   # VectorE pacing-copy length: ~625 ns per link
# First-chunk column-range sub-loads: tile t needs cols <= 16t+17.
SUBS0 = ((0, 34), (34, 66), (66, 130), (130, WP))

_MAX_WAITS = 1


def _split_multi_waits(nc):
    """Split instructions carrying >1 sync-wait into single-wait NOP
    preludes (the walrus build here rejects multi-wait instructions)."""
    counter = 0
    for fn in nc.m.functions:
        for bb in fn.blocks:
            insts = bb.instructions
            i = 0
            while i < len(insts):
                inst = insts[i]
                si = inst.sync_info
                if si is not None and si.on_wait and len(si.on_wait) > _MAX_WAITS:
                    waits = list(si.on_wait)
                    keep = waits[-_MAX_WAITS:]
                    # Reversed: waits satisfied earliest tend to sit at
                    # the front of the list, so putting them in the LAST
                    # prelude NOPs lets the chain decode while the
                    # longest wait (first NOP) is still pending.
                    spill = list(reversed(waits[:-_MAX_WAITS]))
                    nops = []
                    for w in spill:
                        nop = mybir.InstNoOp(
                            name=f"waitsplit_{counter}", ins=[], outs=[]
                        )
                        counter += 1
                        nop.engine = inst.engine
                        nop.sync_info = bass_rust.SyncInfo(on_wait=[w], on_update=[])
                        nops.append(nop)
                    inst.sync_info = bass_rust.SyncInfo(
                        on_wait=keep,
                        on_update=list(si.on_update) if si.on_update else [],
                    )
                    insts[i:i] = nops
                    i += len(nops)
                i += 1
    return counter


def build_nc(bias_total: float):
    nc = bass.Bass("TRN2", target_bir_lowering=False, debug=False)
    x_d = nc.dram_tensor("X", [NBLK, H, WP, IBLK], F16, kind="ExternalInput").ap()
    # Bands pre-transposed on host to [KR, chunk, dw, M] so the load is a
    # straight descriptor-per-partition copy.
    bands_d = nc.dram_tensor("BANDS", [KR, 2, 3, M], F16, kind="ExternalInput").ap()
    y_d = nc.dram_tensor("Y", [NBLK, H, W, IBLK], F16, kind="ExternalOutput").ap()

    with tile.TileContext(nc) as tc:
        with (
            tc.tile_pool(name="const", bufs=1) as cpool,
            tc.tile_pool(name="xin", bufs=5) as xin_pool,
            tc.tile_pool(name="out", bufs=4) as out_pool,
            tc.tile_pool(name="acc", bufs=8, space="PSUM") as psum_pool,
        ):
            # --- PE p-state warm-up, data-independent (memset-fed) -----
            # A tiny first memset gets warm-up 0 onto the PE ~0.5 us in
            # (pinning pe_busy_start early); the remaining warm-ups are
            # paced ~0.7 us apart by VectorE copies so the busy-run never
            # looks idle long enough to reset the ramp before real work.
            pace0 = cpool.tile([KR, M + 2], F16)
            nc.vector.memset(pace0, 0.0)
            wps = psum_pool.tile([M, PSF], F32, tag="ps")
            nc.tensor.matmul(
                wps[:, 0:2], pace0[:, 0:M], pace0[:, 0:2],
                start=True, stop=True,
            )
            pace = cpool.tile([KR, NWARM, PACE_F], F16)
            nc.vector.memset(pace[:, 0, :], 0.0)
            for j in range(NWARM):
                if j > 0:
                    nc.vector.tensor_copy(pace[:, j, :], pace[:, j - 1, :])
                wps = psum_pool.tile([M, PSF], F32, tag="ps")
                nc.tensor.matmul(
                    wps[:, 0:2], pace[:, j, 0:M], pace[:, j, 0:2],
                    start=True, stop=True,
                )

            bands = cpool.tile([KR, 2, 3, M], F16)

            ev = 0
            for bi in range(NBLK):
                for c, (r0, i0) in enumerate(CHUNKS):
                    first = bi == 0 and c == 0
                    is_last = bi == NBLK - 1 and c == len(CHUNKS) - 1
                    xt = xin_pool.tile([KR, WP, IBLK], F16, tag="xt")
                    for wlo, whi in SUBS0 if first else ((0, WP),):
                        nc.sync.dma_start(
                            xt[:, wlo:whi, :],
                            x_d[bi, i0:i0 + KR, wlo:whi, :],
                        )
                        if first and wlo == 0:
                            # Bands on the ACT ring right after the first
                            # X sub-load: their DGE setups overlap.
                            nc.scalar.dma_start(bands, bands_d)
                    xtf = xt.rearrange("k w i -> k (w i)")
                    ot = out_pool.tile([M, W, IBLK], F16, tag="ot")
                    otf = ot.rearrange("m w i -> m (w i)")
                    for t in range(NT):
                        halves = 1
                        hs = PSF // halves
                        for hh in range(halves):
                            ps = psum_pool.tile([M, PSF], F32, tag="ps")
                            if halves == 2:
                                ps = ps[:, 0:hs]
                            for k, dw in enumerate((0, 1, 2)):
                                nc.tensor.matmul(
                                    ps,
                                    bands[:, c, dw, :],
                                    xtf[:, (WT * t + dw) * IBLK + hh * hs:
                                        (WT * t + dw) * IBLK + hh * hs + hs],
                                    start=(k == 0),
                                    stop=(k == 2),
                                )
                            dst = otf[:, t * PSF + hh * hs:
                                      t * PSF + (hh + 1) * hs]
                            if ev % 2 == 0:
                                if bias_total != 0.0:
                                    nc.scalar.activation(
                                        dst,
                                        ps,
                                        mybir.ActivationFunctionType.Copy,
                                        bias=float(bias_total),
                                    )
                                else:
                                    nc.scalar.copy(dst, ps)
                            else:
                                if bias_total != 0.0:
                                    nc.vector.tensor_scalar_add(
                                        dst, ps, float(bias_total)
                                    )
                                else:
                                    nc.vector.tensor_copy(dst, ps)
                            ev += 1
                        # Stores ride the otherwise-idle GPSIMD (SWDGE)
                        # ring so their sem waits never block the load
                        # ring or the eviction engines.  The last chunk
                        # drains in 16-column slices on alternating rings
                        # so the final store chases the final eviction.
                            if is_last and t >= 8:
                                if halves == 2:
                                    wlo = WT * t + hh * (WT // 2)
                                    wn = WT // 2
                                    ring = nc.sync if hh == 1 else nc.scalar
                                else:
                                    wlo = WT * t
                                    wn = WT
                                    ring = (
                                        nc.sync if t == NT - 1
                                        else nc.scalar if t == NT - 2
                                        else nc.gpsimd
                                    )
                                ring.dma_start(
                                    y_d[bi, r0:r0 + M, wlo:wlo + wn, :],
                                    ot[:, wlo:wlo + wn, :],
                                )
                            elif t in (3, 7, 10, 13) and not (is_last and t > 7):
                                wlo = {3: 0, 7: 64, 10: 128, 13: 176}[t]
                                whi = {3: 64, 7: 128, 10: 176, 13: 224}[t]
                                nc.gpsimd.dma_start(
                                    y_d[bi, r0:r0 + M, wlo:whi, :],
                                    ot[:, wlo:whi, :],
                                )
    _split_multi_waits(nc)
    return nc


def build_bands(Keff: np.ndarray) -> np.ndarray:
    """Banded H-contraction matrices, [KR, chunk, dw, M] fp16.

    B[i, c, dw, m] = Keff[dh, dw] where input-row index i corresponds to
    absolute row i0 + i and output row r0 + m needs absolute row
    r0 + m + dh - 1; rows outside [0, H) are dropped (zero padding).
    """
    bands = np.zeros((2, 3, KR, M), dtype=np.float32)
    for c, (r0, i0) in enumerate(CHUNKS):
        for dw in range(3):
            for m in range(M):
                for dh in range(3):
                    arow = r0 + m + dh - 1
                    if 0 <= arow < H:
                        bands[c, dw, arow - i0, m] = Keff[dh, dw]
    return np.ascontiguousarray(bands.transpose(2, 0, 1, 3)).astype(np.float16)


_cache = {}


def kernel(X, K, b, padding, stride) -> np.ndarray:
    X = np.asarray(X, dtype=np.float32)
    K = np.asarray(K, dtype=np.float32)
    b = np.asarray(b, dtype=np.float32)
    assert int(padding) == 1 and int(stride) == 1, (padding, stride)
    bx, cx, hx, wx = X.shape
    assert (bx, cx, hx, wx) == (16, 64, H, W), X.shape

    bk, ck, hk, wk = K.shape
    Keff = K.sum(axis=(0, 1), dtype=np.float32)
    bias_total = float(b.reshape(())) * (bk * ck * hk * wk)

    key = (round(bias_total, 12) != 0.0)
    if key not in _cache:
        _cache[key] = build_nc(bias_total)
    nc = _cache[key]

    bands = build_bands(Keff)
    # Host-side prep: fp16, zero-pad W to 226, lay out per core as
    # [img-block, row, col, img-in-block] so DMA descriptors are
    # contiguous multi-KB runs and matmul window shifts are 64 B-aligned.
    Xf = X.reshape(bx * cx, hx, wx)
    Xp = np.zeros((bx * cx, hx, WP), dtype=np.float16)
    Xp[:, :, 1:1 + W] = Xf
    Xp = Xp.reshape(N_CORES, NBLK, IBLK, hx, WP)
    in_maps = [
        {
            "X": np.ascontiguousarray(Xp[k].transpose(0, 2, 3, 1)),
            "BANDS": bands,
        }
        for k in range(N_CORES)
    ]
    res = run_bass_kernel_spmd(nc, in_maps, core_ids=list(range(N_CORES)))
    # Y comes back [blk, row, col, img] per core -> [imgs, row, col].
    out = np.concatenate(
        [
            np.asarray(r["Y"]).transpose(0, 3, 1, 2).reshape(IMGS, hx, wx)
            for r in res.results
        ],
        axis=0,
    )
    return np.ascontiguousarray(out).astype(np.float32).reshape(bx, cx, hx, wx)
